# revision 1
# baseline (speedup 1.0000x reference)
"""Trainium2 Bass kernel for nn_DA_conv1D (dynamic depthwise conv1d + 1x1 conv
+ channel-attention gate), data-parallel over batch on 8 NeuronCores.

Shapes (hardcoded): x0 [32, 64, 16384] f32, x1 [32, 64] f32.
Each core handles 4 samples, organized as 2 "pairs" of 2 samples so the
128 SBUF partitions hold (2 samples x 64 channels).

Per pair, per 1024-wide tile (SBUF layout [128 part, L free]):
  ps1 = sum_j diag(kern_j) @ x_shift_j   (PE, 2x3 accumulating bf16 matmuls,
                                          N=512 each: PSUM-bank limit)
  lr  = lrelu(ps1)                       (ACT Prelu, PSUM->SBUF, bf16 out)
  ps2 = blockdiag(conv_w) @ lr           (PE, 2x K=128 bf16 matmuls)
  out = x0 * att + ps2                   (DVE scalar_tensor_tensor)
  out += conv_b                          (alternating ACT Identity / DVE
                                          tensor_scalar, to balance engines)

x0 is pre-cast to bf16 on host and DMAed once per chunk (used both for
the matmul path and the residual read; halves input HBM traffic).
The tiny dynamic-weight math (h = lrelu(x1 W1^T), kern = h W2^T, SE gate
att = sigmoid(lrelu(x1 ca_w1^T) ca_w2^T)) is computed on host in fp32 and
shipped as per-core diagonal/gate tensors (a few hundred KB).
"""

import os
import sys

for _p in ("/opt/trn_rl_repo", "/root/.axon_site/_ro/trn_rl_repo"):
    if os.path.isdir(_p) and _p not in sys.path:
        sys.path.append(_p)

import ml_dtypes
import numpy as np

import concourse.bacc as bacc
import concourse.tile as tile
from concourse import mybir
from concourse.bass_utils import run_bass_kernel_spmd

B, C, L, K = 32, 64, 16384, 3
N_CORES = 8
SAMPLES_PER_CORE = B // N_CORES          # 4
PAIRS = SAMPLES_PER_CORE // 2            # 2
P = 128                                  # SBUF partitions = 2 samples x 64 ch
CHUNK = 2048                             # max chunk (SBUF tile size)
# tapered schedule: small chunks at the edges shrink the pipeline
# fill/drain (first matmul waits only on a 0.13 MB DMA; final store is small)
CHUNK_SIZES = [512, 1536] + [2048] * 6 + [1536, 512]
MTILE = 512                              # ACT/DVE tile width
NTILE = 512                              # matmul moving width (PSUM bank)
N_CHUNKS = L // CHUNK
F32 = mybir.dt.float32
BF16 = mybir.dt.bfloat16
BF16_NP = ml_dtypes.bfloat16

TRACE = False          # test harness flips this to profile
USE_LRELU = True       # HW Prelu activation (CoreSim lacks it; see simcheck)
LAST_RESULT = None     # BassKernelResults of the most recent run

_COMPILED = {}         # (use_lrelu,) -> compiled Bacc program


def _build_program(use_lrelu):
    nc = bacc.Bacc("TRN2", target_bir_lowering=False, debug=False,
                   num_devices=N_CORES)

    x0b = nc.dram_tensor("x0b", [PAIRS, P, L], BF16,
                         kind="ExternalInput").ap()
    # diag kernels pre-flattened per partition: [(pair, tap) -> 128 cols]
    diags = nc.dram_tensor("diags", [P, PAIRS * K * P], BF16,
                           kind="ExternalInput").ap()
    # scal[:, 0:PAIRS] = att per pair; scal[:, PAIRS:2*PAIRS] = prelu bias
    # (-sum_j kern_j * d, the depthwise compensation for the host-side
    #  x0 + d shift that folds conv_b into the residual term)
    scal = nc.dram_tensor("scal", [P, 2 * PAIRS], F32,
                          kind="ExternalInput").ap()
    # bf16(d) per pair: halo fill value so padded taps cancel exactly
    dcol = nc.dram_tensor("dcol", [PAIRS, P, 1], BF16,
                          kind="ExternalInput").ap()
    wblk = nc.dram_tensor("wblk", [P, P], BF16, kind="ExternalInput").ap()
    out = nc.dram_tensor("out", [PAIRS, P, L], F32, kind="ExternalOutput").ap()

    mult = mybir.AluOpType.mult
    add = mybir.AluOpType.add
    Relu = mybir.ActivationFunctionType.Relu
    Prelu = mybir.ActivationFunctionType.Prelu
    Ident = mybir.ActivationFunctionType.Identity

    with tile.TileContext(nc) as tc:
        with (
            tc.tile_pool(name="consts", bufs=1) as consts,
            tc.tile_pool(name="xbf", bufs=6) as xbf_pool,
            tc.tile_pool(name="lr", bufs=4) as lr_pool,
            tc.tile_pool(name="r9", bufs=4) as r9_pool,
            tc.tile_pool(name="outc", bufs=4) as out_pool,
            tc.tile_pool(name="ps1", bufs=4, space="PSUM") as ps1_pool,
            tc.tile_pool(name="ps2", bufs=4, space="PSUM") as ps2_pool,
        ):
            # first chunk load issued before the const DMAs so the input
            # stream starts immediately
            sz0 = CHUNK_SIZES[0]
            first_xbf = xbf_pool.tile([P, CHUNK + 4], BF16, tag="xbf")
            nc.sync.dma_start(first_xbf[:, 1:2], dcol[0])
            nc.sync.dma_start(first_xbf[:, 2:sz0 + 3],
                              x0b[0, :, 0:sz0 + 1])

            diag_t = consts.tile([P, PAIRS * K * P], BF16)
            nc.scalar.dma_start(diag_t[:], diags[:])
            wblk_t = consts.tile([P, P], BF16)
            nc.scalar.dma_start(wblk_t[:], wblk[:])
            scal_t = consts.tile([P, 2 * PAIRS], F32)
            nc.scalar.dma_start(scal_t[:], scal[:])
            att = [scal_t[:, p:p + 1] for p in range(PAIRS)]
            pb = [scal_t[:, PAIRS + p:PAIRS + p + 1] for p in range(PAIRS)]

            for p in range(PAIRS):
                lo = 0
                for c, csz in enumerate(CHUNK_SIZES):
                    # xbf[:, i] = x0[lo + i - 2]; i=0 never read
                    if p == 0 and c == 0:
                        xbf = first_xbf
                    else:
                        xbf = xbf_pool.tile([P, CHUNK + 4], BF16, tag="xbf")
                        if c == 0:
                            nc.sync.dma_start(xbf[:, 1:2], dcol[p])
                            nc.sync.dma_start(xbf[:, 2:csz + 3],
                                              x0b[p, :, 0:csz + 1])
                        elif c == len(CHUNK_SIZES) - 1:
                            nc.sync.dma_start(xbf[:, csz + 2:csz + 3],
                                              dcol[p])
                            nc.sync.dma_start(xbf[:, 1:csz + 2],
                                              x0b[p, :, lo - 1:lo + csz])
                        else:
                            nc.sync.dma_start(xbf[:, 1:csz + 3],
                                              x0b[p, :, lo - 1:lo + csz + 1])

                    outc = out_pool.tile([P, CHUNK], F32, tag="outc")
                    for t in range(csz // MTILE):
                        u = t * MTILE
                        ps1 = ps1_pool.tile([P, MTILE], F32)
                        for h in range(MTILE // NTILE):
                            for j in range(K):
                                nc.tensor.matmul(
                                    ps1[:, h * NTILE:(h + 1) * NTILE],
                                    diag_t[:, (p * K + j) * P:
                                           (p * K + j + 1) * P],
                                    xbf[:, u + h * NTILE + 1 + j:
                                        u + h * NTILE + 1 + j + NTILE],
                                    start=(j == 0), stop=(j == K - 1),
                                )
                        lr = lr_pool.tile([P, MTILE], BF16)
                        if use_lrelu:
                            nc.scalar.activation(lr[:], ps1[:], Prelu,
                                                 bias=pb[p], alpha=0.1)
                        else:
                            tt = r9_pool.tile([P, MTILE], F32, tag="tt")
                            nc.scalar.activation(tt[:], ps1[:], Ident,
                                                 bias=pb[p])
                            r9 = r9_pool.tile([P, MTILE], F32)
                            nc.scalar.activation(r9[:], tt[:], Relu,
                                                 scale=0.9)
                            nc.vector.scalar_tensor_tensor(
                                lr[:], tt[:], 0.1, r9[:], op0=mult, op1=add)
                        ps2 = ps2_pool.tile([P, MTILE], F32)
                        for h in range(MTILE // NTILE):
                            nc.tensor.matmul(
                                ps2[:, h * NTILE:(h + 1) * NTILE],
                                wblk_t[:],
                                lr[:, h * NTILE:(h + 1) * NTILE],
                                start=True, stop=True)
                        oslice = outc[:, u:u + MTILE]
                        nc.vector.scalar_tensor_tensor(
                            oslice, xbf[:, u + 2:u + 2 + MTILE],
                            att[p], ps2[:], op0=mult, op1=add)
                    nc.gpsimd.dma_start(out[p, :, lo:lo + csz],
                                        outc[:, :csz])
                    lo += csz

    nc.compile()
    return nc


def _lrelu(x):
    return np.where(x >= 0, x, np.float32(0.1) * x)


def kernel(x0, x1, W1, W2, conv_w, conv_b, ca_w1, ca_w2):
    global LAST_RESULT
    x0 = np.ascontiguousarray(np.asarray(x0, dtype=np.float32))
    x1 = np.asarray(x1, dtype=np.float32)
    W1 = np.asarray(W1, dtype=np.float32)
    W2 = np.asarray(W2, dtype=np.float32)
    conv_w = np.asarray(conv_w, dtype=np.float32)
    conv_b = np.asarray(conv_b, dtype=np.float32)
    ca_w1 = np.asarray(ca_w1, dtype=np.float32)
    ca_w2 = np.asarray(ca_w2, dtype=np.float32)

    # dynamic depthwise kernels + SE gate (tiny, fp32 host math)
    h = _lrelu(x1 @ W1.T)                                   # [B, 64]
    kern = (h @ W2.T).reshape(B, C, K)                      # [B, C, K]
    att = 1.0 / (1.0 + np.exp(-(_lrelu(x1 @ ca_w1.T) @ ca_w2.T)))
    att = att.astype(np.float32)                            # [B, C]

    # block-diagonal 1x1-conv weight as lhsT: lhsT[k, m] = W[m, k]
    wblk_np = np.zeros((P, P), np.float32)
    wblk_np[:C, :C] = conv_w.T
    wblk_np[C:, C:] = conv_w.T
    wblk_np = wblk_np.astype(BF16_NP)

    key = (USE_LRELU,)
    if key not in _COMPILED:
        _COMPILED[key] = _build_program(USE_LRELU)
    nc = _COMPILED[key]

    biasP = np.tile(conv_b, 2).astype(np.float32)            # [P]
    in_maps = []
    for core in range(N_CORES):
        s0 = core * SAMPLES_PER_CORE
        diags_np = np.zeros((P, PAIRS * K * P), np.float32)
        scal_np = np.empty((P, 2 * PAIRS), np.float32)
        dcol_np = np.empty((PAIRS, P, 1), np.float32)
        dvals = np.empty((PAIRS, P), np.float32)
        for p in range(PAIRS):
            ka = kern[s0 + 2 * p]          # [C, K]
            kb = kern[s0 + 2 * p + 1]
            kern_bf = np.empty((P, K), np.float32)
            for j in range(K):
                s = (p * K + j) * P
                d = np.concatenate([ka[:, j], kb[:, j]])
                np.fill_diagonal(diags_np[:, s:s + P], d)
                kern_bf[:, j] = d.astype(BF16_NP).astype(np.float32)
            attp = np.concatenate([att[s0 + 2 * p], att[s0 + 2 * p + 1]])
            dp = biasP / attp                                 # [P]
            dvals[p] = dp
            dcol_np[p, :, 0] = dp
            scal_np[:, p] = attp
            # depthwise compensation: -sum_j bf16(kern_j) * d
            scal_np[:, PAIRS + p] = -(kern_bf.sum(axis=1) * dp)
        x0c = x0[s0:s0 + SAMPLES_PER_CORE].reshape(PAIRS, P, L)
        x0c = (x0c + dvals[:, :, None]).astype(BF16_NP)
        in_maps.append({
            "x0b": x0c,
            "diags": diags_np.astype(BF16_NP),
            "scal": scal_np,
            "dcol": dcol_np.astype(BF16_NP),
            "wblk": wblk_np,
        })

    res = run_bass_kernel_spmd(nc, in_maps, list(range(N_CORES)), trace=TRACE)
    LAST_RESULT = res

    full = np.empty((B, C, L), np.float32)
    for core in range(N_CORES):
        s0 = core * SAMPLES_PER_CORE
        full[s0:s0 + SAMPLES_PER_CORE] = (
            res.results[core]["out"].reshape(SAMPLES_PER_CORE, C, L))
    return full



# revision 2
# speedup vs baseline: 1.0855x; 1.0855x over previous
"""Trainium2 Bass kernel for nn_DA_conv1D (dynamic depthwise conv1d + 1x1 conv
+ channel-attention gate), data-parallel over batch on 8 NeuronCores.

Shapes (hardcoded): x0 [32, 64, 16384] f32, x1 [32, 64] f32.
Each core handles 4 samples, organized as 2 "pairs" of 2 samples so the
128 SBUF partitions hold (2 samples x 64 channels).

Per pair the length axis is processed in 1024-wide groups (2 PSUM banks),
software-pipelined one group deep:
  S1  ps1 = sum_j diag(kern_j) @ x_shift_j   (PE, 6 bf16 matmuls, tap-major
                                              so consecutive MMs share lhsT)
  S2  lr  = lrelu(ps1)                       (ACT Prelu, PSUM->SBUF, bf16)
  S3  ps2 = blockdiag(conv_w) @ lr           (PE, 2 matmuls; issued after the
                                              NEXT group's S1 so the PE never
                                              waits on this group's ACT)
  S4  out = x0 * att + ps2                   (DVE stt, writes bf16)

Output is stored in bf16 (half the store traffic; tolerance is 2e-2) and
upcast to fp32 on host. x0 is pre-cast to bf16 on host and DMAed once per
chunk (used both for the matmul path and the residual read).
The tiny dynamic-weight math (h = lrelu(x1 W1^T), kern = h W2^T, SE gate
att = sigmoid(lrelu(x1 ca_w1^T) ca_w2^T)) is computed on host in fp32 and
shipped as per-core diagonal/gate tensors (a few hundred KB).
"""

import os
import sys

for _p in ("/opt/trn_rl_repo", "/root/.axon_site/_ro/trn_rl_repo"):
    if os.path.isdir(_p) and _p not in sys.path:
        sys.path.append(_p)

import ml_dtypes
import numpy as np

import concourse.bacc as bacc
import concourse.tile as tile
from concourse import mybir
from concourse.bass_utils import run_bass_kernel_spmd

B, C, L, K = 32, 64, 16384, 3
N_CORES = 8
SAMPLES_PER_CORE = B // N_CORES          # 4
PAIRS = SAMPLES_PER_CORE // 2            # 2
P = 128                                  # SBUF partitions = 2 samples x 64 ch
CHUNK = 2048                             # max chunk (SBUF tile size)
# tapered schedule: smaller chunks at the edges shrink the pipeline
# fill/drain (first matmul waits only on a ~0.26 MB DMA; final store is small)
CHUNK_SIZES = [1024] + [2048] * 7 + [1024]
GTILE = 1024                             # ACT/DVE group width (2 PSUM banks)
NTILE = 512                              # matmul moving width (PSUM bank)
F32 = mybir.dt.float32
BF16 = mybir.dt.bfloat16
BF16_NP = ml_dtypes.bfloat16

TRACE = False          # test harness flips this to profile
USE_LRELU = True       # HW Prelu activation (CoreSim lacks it; see simcheck)
LAST_RESULT = None     # BassKernelResults of the most recent run

_COMPILED = {}         # (use_lrelu,) -> compiled Bacc program


def _build_program(use_lrelu):
    nc = bacc.Bacc("TRN2", target_bir_lowering=False, debug=False,
                   num_devices=N_CORES)

    x0b = nc.dram_tensor("x0b", [PAIRS, P, L], BF16,
                         kind="ExternalInput").ap()
    # diag kernels pre-flattened per partition: [(pair, tap) -> 128 cols]
    diags = nc.dram_tensor("diags", [P, PAIRS * K * P], BF16,
                           kind="ExternalInput").ap()
    # scal[:, 0:PAIRS] = att per pair; scal[:, PAIRS:2*PAIRS] = prelu bias
    # (-sum_j kern_j * d, the depthwise compensation for the host-side
    #  x0 + d shift that folds conv_b into the residual term)
    scal = nc.dram_tensor("scal", [P, 2 * PAIRS], F32,
                          kind="ExternalInput").ap()
    # bf16(d) per pair: halo fill value so padded taps cancel exactly
    dcol = nc.dram_tensor("dcol", [PAIRS, P, 1], BF16,
                          kind="ExternalInput").ap()
    wblk = nc.dram_tensor("wblk", [P, P], BF16, kind="ExternalInput").ap()
    out = nc.dram_tensor("out", [PAIRS, P, L], BF16, kind="ExternalOutput").ap()

    mult = mybir.AluOpType.mult
    add = mybir.AluOpType.add
    Relu = mybir.ActivationFunctionType.Relu
    Prelu = mybir.ActivationFunctionType.Prelu
    Ident = mybir.ActivationFunctionType.Identity

    with tile.TileContext(nc) as tc:
        with (
            tc.tile_pool(name="consts", bufs=1) as consts,
            tc.tile_pool(name="xbf", bufs=6) as xbf_pool,
            tc.tile_pool(name="lr", bufs=4) as lr_pool,
            tc.tile_pool(name="r9", bufs=4) as r9_pool,
            tc.tile_pool(name="outc", bufs=4) as out_pool,
            tc.tile_pool(name="ps1", bufs=2, space="PSUM") as ps1_pool,
            tc.tile_pool(name="ps2", bufs=2, space="PSUM") as ps2_pool,
        ):
            # first chunk load issued before the const DMAs so the input
            # stream starts immediately
            sz0 = CHUNK_SIZES[0]
            first_xbf = xbf_pool.tile([P, CHUNK + 4], BF16, tag="xbf")
            nc.sync.dma_start(first_xbf[:, 1:2], dcol[0])
            nc.sync.dma_start(first_xbf[:, 2:sz0 + 3],
                              x0b[0, :, 0:sz0 + 1])

            diag_t = consts.tile([P, PAIRS * K * P], BF16)
            nc.scalar.dma_start(diag_t[:], diags[:])
            wblk_t = consts.tile([P, P], BF16)
            nc.scalar.dma_start(wblk_t[:], wblk[:])
            scal_t = consts.tile([P, 2 * PAIRS], F32)
            nc.scalar.dma_start(scal_t[:], scal[:])
            att = [scal_t[:, p:p + 1] for p in range(PAIRS)]
            pb = [scal_t[:, PAIRS + p:PAIRS + p + 1] for p in range(PAIRS)]

            # flat work list of 1024-wide groups, software-pipelined with a
            # one-group lag for the 1x1-conv + combine + store stages
            prev = None   # (pair, lr, xbf, outc, u, last_in_chunk, dma_args)

            def finish(prev):
                p_, lr_, xbf_, outc_, u_, last_, dma_ = prev
                ps2 = ps2_pool.tile([P, GTILE], F32)
                for h in range(GTILE // NTILE):
                    nc.tensor.matmul(
                        ps2[:, h * NTILE:(h + 1) * NTILE],
                        wblk_t[:],
                        lr_[:, h * NTILE:(h + 1) * NTILE],
                        start=True, stop=True)
                nc.vector.scalar_tensor_tensor(
                    outc_[:, u_:u_ + GTILE], xbf_[:, u_ + 2:u_ + 2 + GTILE],
                    att[p_], ps2[:], op0=mult, op1=add)
                if last_:
                    nc.gpsimd.dma_start(*dma_)

            for p in range(PAIRS):
                lo = 0
                for c, csz in enumerate(CHUNK_SIZES):
                    # xbf[:, i] = x0[lo + i - 2]; i=0 never read
                    if p == 0 and c == 0:
                        xbf = first_xbf
                    else:
                        xbf = xbf_pool.tile([P, CHUNK + 4], BF16, tag="xbf")
                        if c == 0:
                            nc.sync.dma_start(xbf[:, 1:2], dcol[p])
                            nc.sync.dma_start(xbf[:, 2:csz + 3],
                                              x0b[p, :, 0:csz + 1])
                        elif c == len(CHUNK_SIZES) - 1:
                            nc.sync.dma_start(xbf[:, csz + 2:csz + 3],
                                              dcol[p])
                            nc.sync.dma_start(xbf[:, 1:csz + 2],
                                              x0b[p, :, lo - 1:lo + csz])
                        else:
                            nc.sync.dma_start(xbf[:, 1:csz + 3],
                                              x0b[p, :, lo - 1:lo + csz + 1])

                    outc = out_pool.tile([P, CHUNK], BF16, tag="outc")
                    n_groups = csz // GTILE
                    for g in range(n_groups):
                        u = g * GTILE
                        # S1: depthwise matmuls, tap-major (lhsT reuse)
                        ps1 = ps1_pool.tile([P, GTILE], F32)
                        for j in range(K):
                            for h in range(GTILE // NTILE):
                                nc.tensor.matmul(
                                    ps1[:, h * NTILE:(h + 1) * NTILE],
                                    diag_t[:, (p * K + j) * P:
                                           (p * K + j + 1) * P],
                                    xbf[:, u + h * NTILE + 1 + j:
                                        u + h * NTILE + 1 + j + NTILE],
                                    start=(j == 0), stop=(j == K - 1),
                                )
                        # S3/S4/store of the previous group (PE issues the
                        # 1x1 after this group's depthwise, so it never
                        # stalls on the freshly-issued ACT below)
                        if prev is not None:
                            finish(prev)
                        # S2: lrelu
                        lr = lr_pool.tile([P, GTILE], BF16)
                        if use_lrelu:
                            nc.scalar.activation(lr[:], ps1[:], Prelu,
                                                 bias=pb[p], alpha=0.1)
                        else:
                            tt = r9_pool.tile([P, GTILE], F32, tag="tt")
                            nc.scalar.activation(tt[:], ps1[:], Ident,
                                                 bias=pb[p])
                            r9 = r9_pool.tile([P, GTILE], F32)
                            nc.scalar.activation(r9[:], tt[:], Relu,
                                                 scale=0.9)
                            nc.vector.scalar_tensor_tensor(
                                lr[:], tt[:], 0.1, r9[:], op0=mult, op1=add)
                        prev = (p, lr, xbf, outc, u, g == n_groups - 1,
                                (out[p, :, lo:lo + csz], outc[:, :csz]))
                    lo += csz
            finish(prev)

    nc.compile()
    return nc


def _lrelu(x):
    return np.where(x >= 0, x, np.float32(0.1) * x)


def kernel(x0, x1, W1, W2, conv_w, conv_b, ca_w1, ca_w2):
    global LAST_RESULT
    x0 = np.ascontiguousarray(np.asarray(x0, dtype=np.float32))
    x1 = np.asarray(x1, dtype=np.float32)
    W1 = np.asarray(W1, dtype=np.float32)
    W2 = np.asarray(W2, dtype=np.float32)
    conv_w = np.asarray(conv_w, dtype=np.float32)
    conv_b = np.asarray(conv_b, dtype=np.float32)
    ca_w1 = np.asarray(ca_w1, dtype=np.float32)
    ca_w2 = np.asarray(ca_w2, dtype=np.float32)

    # dynamic depthwise kernels + SE gate (tiny, fp32 host math)
    h = _lrelu(x1 @ W1.T)                                   # [B, 64]
    kern = (h @ W2.T).reshape(B, C, K)                      # [B, C, K]
    att = 1.0 / (1.0 + np.exp(-(_lrelu(x1 @ ca_w1.T) @ ca_w2.T)))
    att = att.astype(np.float32)                            # [B, C]

    # block-diagonal 1x1-conv weight as lhsT: lhsT[k, m] = W[m, k]
    wblk_np = np.zeros((P, P), np.float32)
    wblk_np[:C, :C] = conv_w.T
    wblk_np[C:, C:] = conv_w.T
    wblk_np = wblk_np.astype(BF16_NP)

    key = (USE_LRELU,)
    if key not in _COMPILED:
        _COMPILED[key] = _build_program(USE_LRELU)
    nc = _COMPILED[key]

    biasP = np.tile(conv_b, 2).astype(np.float32)            # [P]
    in_maps = []
    for core in range(N_CORES):
        s0 = core * SAMPLES_PER_CORE
        diags_np = np.zeros((P, PAIRS * K * P), np.float32)
        scal_np = np.empty((P, 2 * PAIRS), np.float32)
        dcol_np = np.empty((PAIRS, P, 1), np.float32)
        dvals = np.empty((PAIRS, P), np.float32)
        for p in range(PAIRS):
            ka = kern[s0 + 2 * p]          # [C, K]
            kb = kern[s0 + 2 * p + 1]
            kern_bf = np.empty((P, K), np.float32)
            for j in range(K):
                s = (p * K + j) * P
                d = np.concatenate([ka[:, j], kb[:, j]])
                np.fill_diagonal(diags_np[:, s:s + P], d)
                kern_bf[:, j] = d.astype(BF16_NP).astype(np.float32)
            attp = np.concatenate([att[s0 + 2 * p], att[s0 + 2 * p + 1]])
            dp = biasP / attp                                 # [P]
            dvals[p] = dp
            dcol_np[p, :, 0] = dp
            scal_np[:, p] = attp
            # depthwise compensation: -sum_j bf16(kern_j) * d
            scal_np[:, PAIRS + p] = -(kern_bf.sum(axis=1) * dp)
        x0c = x0[s0:s0 + SAMPLES_PER_CORE].reshape(PAIRS, P, L)
        x0c = (x0c + dvals[:, :, None]).astype(BF16_NP)
        in_maps.append({
            "x0b": x0c,
            "diags": diags_np.astype(BF16_NP),
            "scal": scal_np,
            "dcol": dcol_np.astype(BF16_NP),
            "wblk": wblk_np,
        })

    res = run_bass_kernel_spmd(nc, in_maps, list(range(N_CORES)), trace=TRACE)
    LAST_RESULT = res

    full = np.empty((B, C, L), np.float32)
    for core in range(N_CORES):
        s0 = core * SAMPLES_PER_CORE
        full[s0:s0 + SAMPLES_PER_CORE] = (
            res.results[core]["out"].reshape(SAMPLES_PER_CORE, C, L)
            .astype(np.float32))
    return full


# revision 3
# speedup vs baseline: 1.0943x; 1.0081x over previous
"""Trainium2 Bass kernel for nn_DA_conv1D (dynamic depthwise conv1d + 1x1 conv
+ channel-attention gate), data-parallel over batch on 8 NeuronCores.

Shapes (hardcoded): x0 [32, 64, 16384] f32, x1 [32, 64] f32.
Each core handles 4 samples, organized as 2 "pairs" of 2 samples so the
128 SBUF partitions hold (2 samples x 64 channels).

The length axis is pre-chunked on host into per-(pair, chunk) DRAM tensors
that already include the 1-column halo on each side (edge halos hold the
bias/gate compensation value d so padded taps cancel exactly).  Each chunk
is one contiguous DRAM block -> one large DMA descriptor per queue instead
of 128 x 2KB strided rows.  Chunk sizes taper at the stream edges (pair 0
starts small, pair 1 ends small) to shrink pipeline fill/drain.

Per pair the chunk is processed in <=1024-wide groups (2 PSUM banks),
software-pipelined one group deep:
  S1  ps1 = sum_j diag(kern_j) @ x_shift_j   (PE, bf16 matmuls, tap-major)
  S2  lr  = lrelu(ps1)                       (ACT Prelu, PSUM->SBUF, bf16)
  S3  ps2 = blockdiag(conv_w) @ lr           (PE; issued after the NEXT
                                              group's S1 so the PE never
                                              waits on this group's ACT)
  S4  out = x0 * att + ps2                   (DVE stt, writes bf16)

Output is stored in bf16 (half the store traffic; tolerance is 2e-2) and
upcast to fp32 on host.  The tiny dynamic-weight math (h = lrelu(x1 W1^T),
kern = h W2^T, SE gate att = sigmoid(lrelu(x1 ca_w1^T) ca_w2^T)) is
computed on host in fp32 and shipped as per-core diagonal/gate tensors.
"""

import os
import sys

for _p in ("/opt/trn_rl_repo", "/root/.axon_site/_ro/trn_rl_repo"):
    if os.path.isdir(_p) and _p not in sys.path:
        sys.path.append(_p)

import ml_dtypes
import numpy as np

import concourse.bacc as bacc
import concourse.tile as tile
from concourse import mybir
from concourse.bass_utils import run_bass_kernel_spmd

B, C, L, K = 32, 64, 16384, 3
N_CORES = 8
SAMPLES_PER_CORE = B // N_CORES          # 4
PAIRS = SAMPLES_PER_CORE // 2            # 2
P = 128                                  # SBUF partitions = 2 samples x 64 ch
CHUNK = 2048                             # max chunk (SBUF tile size)
# tapered, asymmetric: pair 0 ramps up (short fill), pair 1 ramps down
# (short drain); the interior runs at the full 2048 chunk size
CHUNK_SIZES = [
    [512, 1536] + [2048] * 7,            # pair 0
    [2048] * 7 + [1536, 512],            # pair 1
]
GTILE = 1024                             # max ACT/DVE group width (2 banks)
NTILE = 512                              # matmul moving width (PSUM bank)
F32 = mybir.dt.float32
BF16 = mybir.dt.bfloat16
BF16_NP = ml_dtypes.bfloat16

TRACE = False          # test harness flips this to profile
USE_LRELU = True       # HW Prelu activation (CoreSim lacks it; see simcheck)
LAST_RESULT = None     # BassKernelResults of the most recent run

_COMPILED = {}         # (use_lrelu,) -> compiled Bacc program


def _groups(csz):
    """Split a chunk into <=GTILE-wide groups."""
    out = []
    u = 0
    while u < csz:
        g = min(GTILE, csz - u)
        out.append((u, g))
        u += g
    return out


def _build_program(use_lrelu):
    nc = bacc.Bacc("TRN2", target_bir_lowering=False, debug=False,
                   num_devices=N_CORES)

    # per-(pair, chunk) input blocks, halo included: col i = x0[lo - 1 + i]
    xin = [[nc.dram_tensor(f"xin_{p}_{c}", [P, csz + 2], BF16,
                           kind="ExternalInput").ap()
            for c, csz in enumerate(CHUNK_SIZES[p])] for p in range(PAIRS)]
    xout = [[nc.dram_tensor(f"out_{p}_{c}", [P, csz], BF16,
                            kind="ExternalOutput").ap()
             for c, csz in enumerate(CHUNK_SIZES[p])] for p in range(PAIRS)]
    # diag kernels pre-flattened per partition: [(pair, tap) -> 128 cols]
    diags = nc.dram_tensor("diags", [P, PAIRS * K * P], BF16,
                           kind="ExternalInput").ap()
    # scal[:, 0:PAIRS] = att per pair; scal[:, PAIRS:2*PAIRS] = prelu bias
    # (-sum_j kern_j * d, the depthwise compensation for the host-side
    #  x0 + d shift that folds conv_b into the residual term)
    scal = nc.dram_tensor("scal", [P, 2 * PAIRS], F32,
                          kind="ExternalInput").ap()
    wblk = nc.dram_tensor("wblk", [P, P], BF16, kind="ExternalInput").ap()

    mult = mybir.AluOpType.mult
    add = mybir.AluOpType.add
    Relu = mybir.ActivationFunctionType.Relu
    Prelu = mybir.ActivationFunctionType.Prelu
    Ident = mybir.ActivationFunctionType.Identity

    with tile.TileContext(nc) as tc:
        with (
            tc.tile_pool(name="consts", bufs=1) as consts,
            tc.tile_pool(name="xbf", bufs=6) as xbf_pool,
            tc.tile_pool(name="lr", bufs=4) as lr_pool,
            tc.tile_pool(name="r9", bufs=4) as r9_pool,
            tc.tile_pool(name="outc", bufs=4) as out_pool,
            tc.tile_pool(name="ps1", bufs=2, space="PSUM") as ps1_pool,
            tc.tile_pool(name="ps2", bufs=2, space="PSUM") as ps2_pool,
        ):
            # first chunk load issued before the const DMAs so the input
            # stream starts immediately
            sz0 = CHUNK_SIZES[0][0]
            first_xbf = xbf_pool.tile([P, CHUNK + 2], BF16, tag="xbf")
            nc.sync.dma_start(first_xbf[:, 0:sz0 + 2], xin[0][0])

            diag_t = consts.tile([P, PAIRS * K * P], BF16)
            nc.scalar.dma_start(diag_t[:], diags[:])
            wblk_t = consts.tile([P, P], BF16)
            nc.scalar.dma_start(wblk_t[:], wblk[:])
            scal_t = consts.tile([P, 2 * PAIRS], F32)
            nc.scalar.dma_start(scal_t[:], scal[:])
            att = [scal_t[:, p:p + 1] for p in range(PAIRS)]
            pb = [scal_t[:, PAIRS + p:PAIRS + p + 1] for p in range(PAIRS)]

            # software pipeline, one group deep, for the 1x1 + combine +
            # store stages
            prev = None   # (pair, lr, xbf, outc, u, gsz, dma_args or None)

            def finish(prev):
                p_, lr_, xbf_, outc_, u_, gsz_, dma_ = prev
                ps2 = ps2_pool.tile([P, GTILE], F32)
                for h in range((gsz_ + NTILE - 1) // NTILE):
                    nc.tensor.matmul(
                        ps2[:, h * NTILE:h * NTILE + min(NTILE, gsz_ - h * NTILE)],
                        wblk_t[:],
                        lr_[:, h * NTILE:h * NTILE + min(NTILE, gsz_ - h * NTILE)],
                        start=True, stop=True)
                nc.vector.scalar_tensor_tensor(
                    outc_[:, u_:u_ + gsz_], xbf_[:, u_ + 1:u_ + 1 + gsz_],
                    att[p_], ps2[:, :gsz_], op0=mult, op1=add)
                if dma_ is not None:
                    nc.gpsimd.dma_start(*dma_)

            for p in range(PAIRS):
                for c, csz in enumerate(CHUNK_SIZES[p]):
                    # xbf[:, i] = x0[lo + i - 1]  (halo pre-packed on host)
                    if p == 0 and c == 0:
                        xbf = first_xbf
                    else:
                        xbf = xbf_pool.tile([P, CHUNK + 2], BF16, tag="xbf")
                        nc.sync.dma_start(xbf[:, 0:csz + 2], xin[p][c])

                    outc = out_pool.tile([P, CHUNK], BF16, tag="outc")
                    groups = _groups(csz)
                    for gi, (u, gsz) in enumerate(groups):
                        # S1: depthwise matmuls, tap-major (lhsT reuse)
                        ps1 = ps1_pool.tile([P, GTILE], F32)
                        for j in range(K):
                            for h in range((gsz + NTILE - 1) // NTILE):
                                n = min(NTILE, gsz - h * NTILE)
                                nc.tensor.matmul(
                                    ps1[:, h * NTILE:h * NTILE + n],
                                    diag_t[:, (p * K + j) * P:
                                           (p * K + j + 1) * P],
                                    xbf[:, u + h * NTILE + j:
                                        u + h * NTILE + j + n],
                                    start=(j == 0), stop=(j == K - 1),
                                )
                        # S3/S4/store of the previous group (PE issues the
                        # 1x1 after this group's depthwise, so it never
                        # stalls on the freshly-issued ACT below)
                        if prev is not None:
                            finish(prev)
                        # S2: lrelu
                        lr = lr_pool.tile([P, GTILE], BF16)
                        if use_lrelu:
                            nc.scalar.activation(lr[:, :gsz], ps1[:, :gsz],
                                                 Prelu, bias=pb[p], alpha=0.1)
                        else:
                            tt = r9_pool.tile([P, GTILE], F32, tag="tt")
                            nc.scalar.activation(tt[:, :gsz], ps1[:, :gsz],
                                                 Ident, bias=pb[p])
                            r9 = r9_pool.tile([P, GTILE], F32)
                            nc.scalar.activation(r9[:, :gsz], tt[:, :gsz],
                                                 Relu, scale=0.9)
                            nc.vector.scalar_tensor_tensor(
                                lr[:, :gsz], tt[:, :gsz], 0.1, r9[:, :gsz],
                                op0=mult, op1=add)
                        dma = None
                        if gi == len(groups) - 1:
                            dma = (xout[p][c], outc[:, :csz])
                        prev = (p, lr, xbf, outc, u, gsz, dma)
            finish(prev)

    nc.compile()
    return nc


def _lrelu(x):
    return np.where(x >= 0, x, np.float32(0.1) * x)


def kernel(x0, x1, W1, W2, conv_w, conv_b, ca_w1, ca_w2):
    global LAST_RESULT
    x0 = np.ascontiguousarray(np.asarray(x0, dtype=np.float32))
    x1 = np.asarray(x1, dtype=np.float32)
    W1 = np.asarray(W1, dtype=np.float32)
    W2 = np.asarray(W2, dtype=np.float32)
    conv_w = np.asarray(conv_w, dtype=np.float32)
    conv_b = np.asarray(conv_b, dtype=np.float32)
    ca_w1 = np.asarray(ca_w1, dtype=np.float32)
    ca_w2 = np.asarray(ca_w2, dtype=np.float32)

    # dynamic depthwise kernels + SE gate (tiny, fp32 host math)
    h = _lrelu(x1 @ W1.T)                                   # [B, 64]
    kern = (h @ W2.T).reshape(B, C, K)                      # [B, C, K]
    att = 1.0 / (1.0 + np.exp(-(_lrelu(x1 @ ca_w1.T) @ ca_w2.T)))
    att = att.astype(np.float32)                            # [B, C]

    # block-diagonal 1x1-conv weight as lhsT: lhsT[k, m] = W[m, k]
    wblk_np = np.zeros((P, P), np.float32)
    wblk_np[:C, :C] = conv_w.T
    wblk_np[C:, C:] = conv_w.T
    wblk_np = wblk_np.astype(BF16_NP)

    key = (USE_LRELU,)
    if key not in _COMPILED:
        _COMPILED[key] = _build_program(USE_LRELU)
    nc = _COMPILED[key]

    biasP = np.tile(conv_b, 2).astype(np.float32)            # [P]
    in_maps = []
    for core in range(N_CORES):
        s0 = core * SAMPLES_PER_CORE
        diags_np = np.zeros((P, PAIRS * K * P), np.float32)
        scal_np = np.empty((P, 2 * PAIRS), np.float32)
        in_map = {}
        for p in range(PAIRS):
            ka = kern[s0 + 2 * p]          # [C, K]
            kb = kern[s0 + 2 * p + 1]
            kern_bf = np.empty((P, K), np.float32)
            for j in range(K):
                s = (p * K + j) * P
                d = np.concatenate([ka[:, j], kb[:, j]])
                np.fill_diagonal(diags_np[:, s:s + P], d)
                kern_bf[:, j] = d.astype(BF16_NP).astype(np.float32)
            attp = np.concatenate([att[s0 + 2 * p], att[s0 + 2 * p + 1]])
            dp = biasP / attp                                 # [P]
            scal_np[:, p] = attp
            # depthwise compensation: -sum_j bf16(kern_j) * d
            scal_np[:, PAIRS + p] = -(kern_bf.sum(axis=1) * dp)
            # shifted input for this pair, with halo columns; edge halo = d
            xp = x0[s0 + 2 * p:s0 + 2 * p + 2].reshape(P, L) + dp[:, None]
            xp8 = np.empty((P, L + 2), BF16_NP)
            xp8[:, 1:L + 1] = xp.astype(BF16_NP)
            dp8 = dp.astype(BF16_NP)
            xp8[:, 0] = dp8
            xp8[:, L + 1] = dp8
            lo = 0
            for c, csz in enumerate(CHUNK_SIZES[p]):
                in_map[f"xin_{p}_{c}"] = np.ascontiguousarray(
                    xp8[:, lo:lo + csz + 2])
                lo += csz
        in_map["diags"] = diags_np.astype(BF16_NP)
        in_map["scal"] = scal_np
        in_map["wblk"] = wblk_np
        in_maps.append(in_map)

    res = run_bass_kernel_spmd(nc, in_maps, list(range(N_CORES)), trace=TRACE)
    LAST_RESULT = res

    full = np.empty((B, C, L), np.float32)
    for core in range(N_CORES):
        s0 = core * SAMPLES_PER_CORE
        r = res.results[core]
        for p in range(PAIRS):
            cols = np.concatenate(
                [r[f"out_{p}_{c}"] for c in range(len(CHUNK_SIZES[p]))],
                axis=1)
            full[s0 + 2 * p:s0 + 2 * p + 2] = (
                cols.reshape(2, C, L).astype(np.float32))
    return full


# revision 6
# speedup vs baseline: 1.1004x; 1.0056x over previous
"""Trainium2 Bass kernel for nn_DA_conv1D (dynamic depthwise conv1d + 1x1 conv
+ channel-attention gate), data-parallel over batch on 8 NeuronCores.

Shapes (hardcoded): x0 [32, 64, 16384] f32, x1 [32, 64] f32.
Each core handles 4 samples, organized as 2 "pairs" of 2 samples so the
128 SBUF partitions hold (2 samples x 64 channels).

The length axis is pre-chunked on host into per-(pair, chunk) DRAM tensors
that already include the 1-column halo on each side (edge halos hold the
bias/gate compensation value d so padded taps cancel exactly).  Each chunk
is one contiguous DRAM block -> one large DMA descriptor per queue instead
of 128 x 2KB strided rows.  Chunk sizes taper at the stream edges (pair 0
starts small, pair 1 ends small) to shrink pipeline fill/drain.

Per pair the chunk is processed in <=1024-wide groups (2 PSUM banks),
software-pipelined one group deep:
  S1  ps1 = sum_j diag(kern_j) @ x_shift_j   (PE, bf16 matmuls, tap-major)
  S2  lr  = lrelu(ps1)                       (ACT Prelu, PSUM->SBUF, bf16)
  S3  ps2 = blockdiag(conv_w) @ lr           (PE; issued after the NEXT
                                              group's S1 so the PE never
                                              waits on this group's ACT)
  S4  out = x0 * att + ps2                   (DVE stt, writes bf16)

Output is stored in bf16 (half the store traffic; tolerance is 2e-2) and
upcast to fp32 on host.  The tiny dynamic-weight math (h = lrelu(x1 W1^T),
kern = h W2^T, SE gate att = sigmoid(lrelu(x1 ca_w1^T) ca_w2^T)) is
computed on host in fp32 and shipped as per-core diagonal/gate tensors.
"""

import os
import sys

for _p in ("/opt/trn_rl_repo", "/root/.axon_site/_ro/trn_rl_repo"):
    if os.path.isdir(_p) and _p not in sys.path:
        sys.path.append(_p)

import ml_dtypes
import numpy as np

import concourse.bacc as bacc
import concourse.tile as tile
from concourse import mybir
from concourse.bass_utils import run_bass_kernel_spmd

B, C, L, K = 32, 64, 16384, 3
N_CORES = 8
SAMPLES_PER_CORE = B // N_CORES          # 4
PAIRS = SAMPLES_PER_CORE // 2            # 2
P = 128                                  # SBUF partitions = 2 samples x 64 ch
CHUNK = 2048                             # max chunk (SBUF tile size)
# tapered, asymmetric: pair 0 ramps up (short fill), pair 1 ramps down
# (short drain); the interior runs at the full 2048 chunk size
CHUNK_SIZES = [
    [512, 1536] + [2048] * 7,            # pair 0
    [2048] * 7 + [1536, 256, 256],       # pair 1
]
GTILE = 1024                             # max ACT/DVE group width (2 banks)
NTILE = 512                              # matmul moving width (PSUM bank)
F32 = mybir.dt.float32
BF16 = mybir.dt.bfloat16
BF16_NP = ml_dtypes.bfloat16

TRACE = False          # test harness flips this to profile
USE_LRELU = True       # HW Prelu activation (CoreSim lacks it; see simcheck)
LAST_RESULT = None     # BassKernelResults of the most recent run

_COMPILED = {}         # (use_lrelu,) -> compiled Bacc program


def _groups(csz):
    """Split a chunk into <=GTILE-wide groups."""
    out = []
    u = 0
    while u < csz:
        g = min(GTILE, csz - u)
        out.append((u, g))
        u += g
    return out


def _build_program(use_lrelu):
    nc = bacc.Bacc("TRN2", target_bir_lowering=False, debug=False,
                   num_devices=N_CORES)

    # per-(pair, chunk) input blocks, halo included: col i = x0[lo - 1 + i]
    xin = [[nc.dram_tensor(f"xin_{p}_{c}", [P, csz + 2], BF16,
                           kind="ExternalInput").ap()
            for c, csz in enumerate(CHUNK_SIZES[p])] for p in range(PAIRS)]
    xout = [[nc.dram_tensor(f"out_{p}_{c}", [P, csz], BF16,
                            kind="ExternalOutput").ap()
             for c, csz in enumerate(CHUNK_SIZES[p])] for p in range(PAIRS)]
    # diag kernels pre-flattened per partition: [(pair, tap) -> 128 cols]
    diags = nc.dram_tensor("diags", [P, PAIRS * K * P], BF16,
                           kind="ExternalInput").ap()
    # scal[:, 0:PAIRS] = att per pair; scal[:, PAIRS:2*PAIRS] = prelu bias
    # (-sum_j kern_j * d, the depthwise compensation for the host-side
    #  x0 + d shift that folds conv_b into the residual term)
    scal = nc.dram_tensor("scal", [P, 2 * PAIRS], F32,
                          kind="ExternalInput").ap()
    wblk = nc.dram_tensor("wblk", [P, P], BF16, kind="ExternalInput").ap()

    mult = mybir.AluOpType.mult
    add = mybir.AluOpType.add
    Relu = mybir.ActivationFunctionType.Relu
    Prelu = mybir.ActivationFunctionType.Prelu
    Ident = mybir.ActivationFunctionType.Identity

    with tile.TileContext(nc) as tc:
        with (
            tc.tile_pool(name="consts", bufs=1) as consts,
            tc.tile_pool(name="xbf", bufs=6) as xbf_pool,
            tc.tile_pool(name="lr", bufs=4) as lr_pool,
            tc.tile_pool(name="r9", bufs=4) as r9_pool,
            tc.tile_pool(name="outc", bufs=4) as out_pool,
            tc.tile_pool(name="ps1", bufs=2, space="PSUM") as ps1_pool,
            tc.tile_pool(name="ps2", bufs=2, space="PSUM") as ps2_pool,
        ):
            # first chunk load issued before the const DMAs so the input
            # stream starts immediately
            sz0 = CHUNK_SIZES[0][0]
            first_xbf = xbf_pool.tile([P, CHUNK + 2], BF16, tag="xbf")
            nc.sync.dma_start(first_xbf[:, 0:sz0 + 2], xin[0][0])

            # diag_t goes via the (otherwise idle) gpsimd queue: the scalar
            # queue stalls ~1.3us on the auto-inserted ACT_TABLE_LOAD, which
            # would delay the first depthwise matmul (it needs diag_t)
            diag_t = consts.tile([P, PAIRS * K * P], BF16)
            nc.gpsimd.dma_start(diag_t[:], diags[:])
            wblk_t = consts.tile([P, P], BF16)
            nc.scalar.dma_start(wblk_t[:], wblk[:])
            scal_t = consts.tile([P, 2 * PAIRS], F32)
            nc.scalar.dma_start(scal_t[:], scal[:])
            att = [scal_t[:, p:p + 1] for p in range(PAIRS)]
            pb = [scal_t[:, PAIRS + p:PAIRS + p + 1] for p in range(PAIRS)]

            # software pipeline, one group deep, for the 1x1 + combine +
            # store stages
            prev = None   # (pair, lr, xbf, outc, u, gsz, dma_args or None)

            def finish(prev):
                p_, lr_, xbf_, outc_, u_, gsz_, dma_ = prev
                ps2 = ps2_pool.tile([P, GTILE], F32)
                for h in range((gsz_ + NTILE - 1) // NTILE):
                    nc.tensor.matmul(
                        ps2[:, h * NTILE:h * NTILE + min(NTILE, gsz_ - h * NTILE)],
                        wblk_t[:],
                        lr_[:, h * NTILE:h * NTILE + min(NTILE, gsz_ - h * NTILE)],
                        start=True, stop=True)
                nc.vector.scalar_tensor_tensor(
                    outc_[:, u_:u_ + gsz_], xbf_[:, u_ + 1:u_ + 1 + gsz_],
                    att[p_], ps2[:, :gsz_], op0=mult, op1=add)
                if dma_ is not None:
                    nc.gpsimd.dma_start(*dma_)

            for p in range(PAIRS):
                for c, csz in enumerate(CHUNK_SIZES[p]):
                    # xbf[:, i] = x0[lo + i - 1]  (halo pre-packed on host)
                    if p == 0 and c == 0:
                        xbf = first_xbf
                    else:
                        xbf = xbf_pool.tile([P, CHUNK + 2], BF16, tag="xbf")
                        nc.sync.dma_start(xbf[:, 0:csz + 2], xin[p][c])

                    outc = out_pool.tile([P, CHUNK], BF16, tag="outc")
                    groups = _groups(csz)
                    for gi, (u, gsz) in enumerate(groups):
                        # S1: depthwise matmuls, tap-major (lhsT reuse)
                        ps1 = ps1_pool.tile([P, GTILE], F32)
                        for j in range(K):
                            for h in range((gsz + NTILE - 1) // NTILE):
                                n = min(NTILE, gsz - h * NTILE)
                                nc.tensor.matmul(
                                    ps1[:, h * NTILE:h * NTILE + n],
                                    diag_t[:, (p * K + j) * P:
                                           (p * K + j + 1) * P],
                                    xbf[:, u + h * NTILE + j:
                                        u + h * NTILE + j + n],
                                    start=(j == 0), stop=(j == K - 1),
                                )
                        # S3/S4/store of the previous group (PE issues the
                        # 1x1 after this group's depthwise, so it never
                        # stalls on the freshly-issued ACT below)
                        if prev is not None:
                            finish(prev)
                        # S2: lrelu
                        lr = lr_pool.tile([P, GTILE], BF16)
                        if use_lrelu:
                            nc.scalar.activation(lr[:, :gsz], ps1[:, :gsz],
                                                 Prelu, bias=pb[p], alpha=0.1)
                        else:
                            tt = r9_pool.tile([P, GTILE], F32, tag="tt")
                            nc.scalar.activation(tt[:, :gsz], ps1[:, :gsz],
                                                 Ident, bias=pb[p])
                            r9 = r9_pool.tile([P, GTILE], F32)
                            nc.scalar.activation(r9[:, :gsz], tt[:, :gsz],
                                                 Relu, scale=0.9)
                            nc.vector.scalar_tensor_tensor(
                                lr[:, :gsz], tt[:, :gsz], 0.1, r9[:, :gsz],
                                op0=mult, op1=add)
                        dma = None
                        if gi == len(groups) - 1:
                            dma = (xout[p][c], outc[:, :csz])
                        prev = (p, lr, xbf, outc, u, gsz, dma)
            finish(prev)

    nc.compile()
    return nc


def _lrelu(x):
    return np.where(x >= 0, x, np.float32(0.1) * x)


def kernel(x0, x1, W1, W2, conv_w, conv_b, ca_w1, ca_w2):
    global LAST_RESULT
    x0 = np.ascontiguousarray(np.asarray(x0, dtype=np.float32))
    x1 = np.asarray(x1, dtype=np.float32)
    W1 = np.asarray(W1, dtype=np.float32)
    W2 = np.asarray(W2, dtype=np.float32)
    conv_w = np.asarray(conv_w, dtype=np.float32)
    conv_b = np.asarray(conv_b, dtype=np.float32)
    ca_w1 = np.asarray(ca_w1, dtype=np.float32)
    ca_w2 = np.asarray(ca_w2, dtype=np.float32)

    # dynamic depthwise kernels + SE gate (tiny, fp32 host math)
    h = _lrelu(x1 @ W1.T)                                   # [B, 64]
    kern = (h @ W2.T).reshape(B, C, K)                      # [B, C, K]
    att = 1.0 / (1.0 + np.exp(-(_lrelu(x1 @ ca_w1.T) @ ca_w2.T)))
    att = att.astype(np.float32)                            # [B, C]

    # block-diagonal 1x1-conv weight as lhsT: lhsT[k, m] = W[m, k]
    wblk_np = np.zeros((P, P), np.float32)
    wblk_np[:C, :C] = conv_w.T
    wblk_np[C:, C:] = conv_w.T
    wblk_np = wblk_np.astype(BF16_NP)

    key = (USE_LRELU,)
    if key not in _COMPILED:
        _COMPILED[key] = _build_program(USE_LRELU)
    nc = _COMPILED[key]

    biasP = np.tile(conv_b, 2).astype(np.float32)            # [P]
    in_maps = []
    for core in range(N_CORES):
        s0 = core * SAMPLES_PER_CORE
        diags_np = np.zeros((P, PAIRS * K * P), np.float32)
        scal_np = np.empty((P, 2 * PAIRS), np.float32)
        in_map = {}
        for p in range(PAIRS):
            ka = kern[s0 + 2 * p]          # [C, K]
            kb = kern[s0 + 2 * p + 1]
            kern_bf = np.empty((P, K), np.float32)
            for j in range(K):
                s = (p * K + j) * P
                d = np.concatenate([ka[:, j], kb[:, j]])
                np.fill_diagonal(diags_np[:, s:s + P], d)
                kern_bf[:, j] = d.astype(BF16_NP).astype(np.float32)
            attp = np.concatenate([att[s0 + 2 * p], att[s0 + 2 * p + 1]])
            dp = biasP / attp                                 # [P]
            scal_np[:, p] = attp
            # depthwise compensation: -sum_j bf16(kern_j) * d
            scal_np[:, PAIRS + p] = -(kern_bf.sum(axis=1) * dp)
            # shifted input for this pair, with halo columns; edge halo = d
            xp = x0[s0 + 2 * p:s0 + 2 * p + 2].reshape(P, L) + dp[:, None]
            xp8 = np.empty((P, L + 2), BF16_NP)
            xp8[:, 1:L + 1] = xp.astype(BF16_NP)
            dp8 = dp.astype(BF16_NP)
            xp8[:, 0] = dp8
            xp8[:, L + 1] = dp8
            lo = 0
            for c, csz in enumerate(CHUNK_SIZES[p]):
                in_map[f"xin_{p}_{c}"] = np.ascontiguousarray(
                    xp8[:, lo:lo + csz + 2])
                lo += csz
        in_map["diags"] = diags_np.astype(BF16_NP)
        in_map["scal"] = scal_np
        in_map["wblk"] = wblk_np
        in_maps.append(in_map)

    res = run_bass_kernel_spmd(nc, in_maps, list(range(N_CORES)), trace=TRACE)
    LAST_RESULT = res

    full = np.empty((B, C, L), np.float32)
    for core in range(N_CORES):
        s0 = core * SAMPLES_PER_CORE
        r = res.results[core]
        for p in range(PAIRS):
            cols = np.concatenate(
                [r[f"out_{p}_{c}"] for c in range(len(CHUNK_SIZES[p]))],
                axis=1)
            full[s0 + 2 * p:s0 + 2 * p + 2] = (
                cols.reshape(2, C, L).astype(np.float32))
    return full


# revision 8
# speedup vs baseline: 1.1228x; 1.0204x over previous
"""Trainium2 Bass kernel for nn_DA_conv1D (dynamic depthwise conv1d + 1x1 conv
+ channel-attention gate), data-parallel over batch on 8 NeuronCores.

Shapes (hardcoded): x0 [32, 64, 16384] f32, x1 [32, 64] f32.
Each core handles 4 samples, organized as 2 "pairs" of 2 samples so the
128 SBUF partitions hold (2 samples x 64 channels).

The length axis is pre-chunked on host into per-(pair, chunk) DRAM tensors
that already include the 1-column halo on each side (edge halos hold the
bias/gate compensation value d so padded taps cancel exactly).  Each chunk
is one contiguous DRAM block -> one large DMA descriptor per queue instead
of 128 x 2KB strided rows.  Chunk sizes taper at the stream edges (pair 0
starts small, pair 1 ends small) to shrink pipeline fill/drain.

Per pair the chunk is processed in <=1024-wide groups (2 PSUM banks),
software-pipelined one group deep:
  S1  ps1 = sum_j diag(kern_j) @ x_shift_j   (PE, bf16 matmuls, tap-major)
  S2  lr  = lrelu(ps1)                       (ACT Prelu, PSUM->SBUF, bf16)
  S3  ps2 = blockdiag(conv_w) @ lr           (PE; issued after the NEXT
                                              group's S1 so the PE never
                                              waits on this group's ACT)
  S4  out = x0 * att + ps2                   (DVE stt, writes bf16)

Output is stored in bf16 (half the store traffic; tolerance is 2e-2) and
upcast to fp32 on host.  The tiny dynamic-weight math (h = lrelu(x1 W1^T),
kern = h W2^T, SE gate att = sigmoid(lrelu(x1 ca_w1^T) ca_w2^T)) is
computed on host in fp32 and shipped as per-core diagonal/gate tensors.
"""

import os
import sys

for _p in ("/opt/trn_rl_repo", "/root/.axon_site/_ro/trn_rl_repo"):
    if os.path.isdir(_p) and _p not in sys.path:
        sys.path.append(_p)

import ml_dtypes
import numpy as np

import concourse.bacc as bacc
import concourse.tile as tile
from concourse import mybir
from concourse.bass_utils import run_bass_kernel_spmd

B, C, L, K = 32, 64, 16384, 3
N_CORES = 8
SAMPLES_PER_CORE = B // N_CORES          # 4
PAIRS = SAMPLES_PER_CORE // 2            # 2
P = 128                                  # SBUF partitions = 2 samples x 64 ch
CHUNK = 2048                             # max chunk (SBUF tile size)
# tapered, asymmetric: pair 0 ramps up (short fill), pair 1 ramps down
# (short drain); the interior runs at the full 2048 chunk size
CHUNK_SIZES = [
    [512, 1536] + [2048] * 7,            # pair 0
    [2048] * 7 + [1536, 256, 256],       # pair 1
]
GTILE = 1024                             # max ACT/DVE group width (2 banks)
NTILE = 512                              # matmul moving width (PSUM bank)
F32 = mybir.dt.float32
BF16 = mybir.dt.bfloat16
BF16_NP = ml_dtypes.bfloat16

TRACE = False          # test harness flips this to profile
USE_LRELU = True       # HW Prelu activation (CoreSim lacks it; see simcheck)
LAST_RESULT = None     # BassKernelResults of the most recent run

_COMPILED = {}         # (use_lrelu,) -> compiled Bacc program


def _groups(csz):
    """Split a chunk into <=GTILE-wide groups."""
    out = []
    u = 0
    while u < csz:
        g = min(GTILE, csz - u)
        out.append((u, g))
        u += g
    return out


def _build_program(use_lrelu):
    nc = bacc.Bacc("TRN2", target_bir_lowering=False, debug=False,
                   num_devices=N_CORES)

    # per-(pair, chunk) input blocks, halo included: col i = x0[lo - 1 + i]
    xin = [[nc.dram_tensor(f"xin_{p}_{c}", [P, csz + 2], BF16,
                           kind="ExternalInput").ap()
            for c, csz in enumerate(CHUNK_SIZES[p])] for p in range(PAIRS)]
    xout = [[nc.dram_tensor(f"out_{p}_{c}", [P, csz], BF16,
                            kind="ExternalOutput").ap()
             for c, csz in enumerate(CHUNK_SIZES[p])] for p in range(PAIRS)]
    # diag kernels pre-flattened per partition: [(pair, tap) -> 128 cols]
    diags = nc.dram_tensor("diags", [P, PAIRS * K * P], BF16,
                           kind="ExternalInput").ap()
    # scal[:, 0:PAIRS] = att per pair; scal[:, PAIRS:2*PAIRS] = prelu bias
    # (-sum_j kern_j * d, the depthwise compensation for the host-side
    #  x0 + d shift that folds conv_b into the residual term)
    scal = nc.dram_tensor("scal", [P, 2 * PAIRS], F32,
                          kind="ExternalInput").ap()
    wblk = nc.dram_tensor("wblk", [P, P], BF16, kind="ExternalInput").ap()

    mult = mybir.AluOpType.mult
    add = mybir.AluOpType.add
    Relu = mybir.ActivationFunctionType.Relu
    Prelu = mybir.ActivationFunctionType.Prelu
    Ident = mybir.ActivationFunctionType.Identity

    with tile.TileContext(nc) as tc:
        with (
            tc.tile_pool(name="consts", bufs=1) as consts,
            tc.tile_pool(name="xbf", bufs=6) as xbf_pool,
            tc.tile_pool(name="lr", bufs=4) as lr_pool,
            tc.tile_pool(name="outc", bufs=4) as out_pool,
            tc.tile_pool(name="ps1", bufs=2, space="PSUM") as ps1_pool,
            tc.tile_pool(name="ps2", bufs=2, space="PSUM") as ps2_pool,
        ):
            # diag_t + first chunk first on the sync queue (both gate the
            # first depthwise matmul); remaining consts go to the scalar
            # queue, whose stream stalls ~1.3us on the auto-inserted
            # ACT_TABLE_LOAD (fine: wblk/scal are needed later)
            diag_t = consts.tile([P, PAIRS * K * P], BF16)
            nc.sync.dma_start(diag_t[:], diags[:])
            sz0 = CHUNK_SIZES[0][0]
            first_xbf = xbf_pool.tile([P, CHUNK + 2], BF16, tag="xbf")
            nc.sync.dma_start(first_xbf[:, 0:sz0 + 2], xin[0][0])

            wblk_t = consts.tile([P, P], BF16)
            nc.scalar.dma_start(wblk_t[:], wblk[:])
            scal_t = consts.tile([P, 2 * PAIRS], F32)
            nc.scalar.dma_start(scal_t[:], scal[:])
            att = [scal_t[:, p:p + 1] for p in range(PAIRS)]
            pb = [scal_t[:, PAIRS + p:PAIRS + p + 1] for p in range(PAIRS)]

            # software pipeline, one group deep, for the 1x1 + combine +
            # store stages.  Output stores ride the sync queue too (no
            # gpsimd SWDGE): each store is issued right AFTER the next
            # chunk's input DMA so its wait (on that chunk's combine) never
            # delays input prefetch.
            prev = None   # (pair, lr, xbf, outc, u, gsz, dma_args or None)
            pending_store = []

            def finish(prev):
                p_, lr_, xbf_, outc_, u_, gsz_, dma_ = prev
                ps2 = ps2_pool.tile([P, GTILE], F32)
                for h in range((gsz_ + NTILE - 1) // NTILE):
                    nc.tensor.matmul(
                        ps2[:, h * NTILE:h * NTILE + min(NTILE, gsz_ - h * NTILE)],
                        wblk_t[:],
                        lr_[:, h * NTILE:h * NTILE + min(NTILE, gsz_ - h * NTILE)],
                        start=True, stop=True)
                nc.vector.scalar_tensor_tensor(
                    outc_[:, u_:u_ + gsz_], xbf_[:, u_ + 1:u_ + 1 + gsz_],
                    att[p_], ps2[:, :gsz_], op0=mult, op1=add)
                if dma_ is not None:
                    pending_store.append(dma_)

            for p in range(PAIRS):
                for c, csz in enumerate(CHUNK_SIZES[p]):
                    # xbf[:, i] = x0[lo + i - 1]  (halo pre-packed on host)
                    if p == 0 and c == 0:
                        xbf = first_xbf
                    else:
                        xbf = xbf_pool.tile([P, CHUNK + 2], BF16, tag="xbf")
                        nc.sync.dma_start(xbf[:, 0:csz + 2], xin[p][c])
                        while pending_store:
                            nc.sync.dma_start(*pending_store.pop(0))

                    outc = out_pool.tile([P, CHUNK], BF16, tag="outc")
                    groups = _groups(csz)
                    for gi, (u, gsz) in enumerate(groups):
                        # S1: depthwise matmuls, tap-major (lhsT reuse)
                        ps1 = ps1_pool.tile([P, GTILE], F32)
                        for j in range(K):
                            for h in range((gsz + NTILE - 1) // NTILE):
                                n = min(NTILE, gsz - h * NTILE)
                                nc.tensor.matmul(
                                    ps1[:, h * NTILE:h * NTILE + n],
                                    diag_t[:, (p * K + j) * P:
                                           (p * K + j + 1) * P],
                                    xbf[:, u + h * NTILE + j:
                                        u + h * NTILE + j + n],
                                    start=(j == 0), stop=(j == K - 1),
                                )
                        # S3/S4/store of the previous group (PE issues the
                        # 1x1 after this group's depthwise, so it never
                        # stalls on the freshly-issued ACT below)
                        if prev is not None:
                            finish(prev)
                        # S2: lrelu
                        lr = lr_pool.tile([P, GTILE], BF16)
                        if use_lrelu:
                            nc.scalar.activation(lr[:, :gsz], ps1[:, :gsz],
                                                 Prelu, bias=pb[p], alpha=0.1)
                        else:
                            tt = lr_pool.tile([P, GTILE], F32, tag="tt")
                            nc.scalar.activation(tt[:, :gsz], ps1[:, :gsz],
                                                 Ident, bias=pb[p])
                            r9 = lr_pool.tile([P, GTILE], F32, tag="r9")
                            nc.scalar.activation(r9[:, :gsz], tt[:, :gsz],
                                                 Relu, scale=0.9)
                            nc.vector.scalar_tensor_tensor(
                                lr[:, :gsz], tt[:, :gsz], 0.1, r9[:, :gsz],
                                op0=mult, op1=add)
                        dma = None
                        if gi == len(groups) - 1:
                            dma = (xout[p][c], outc[:, :csz])
                        prev = (p, lr, xbf, outc, u, gsz, dma)
            finish(prev)
            while pending_store:
                nc.sync.dma_start(*pending_store.pop(0))

    nc.compile()
    return nc


def _lrelu(x):
    return np.where(x >= 0, x, np.float32(0.1) * x)


def kernel(x0, x1, W1, W2, conv_w, conv_b, ca_w1, ca_w2):
    global LAST_RESULT
    x0 = np.ascontiguousarray(np.asarray(x0, dtype=np.float32))
    x1 = np.asarray(x1, dtype=np.float32)
    W1 = np.asarray(W1, dtype=np.float32)
    W2 = np.asarray(W2, dtype=np.float32)
    conv_w = np.asarray(conv_w, dtype=np.float32)
    conv_b = np.asarray(conv_b, dtype=np.float32)
    ca_w1 = np.asarray(ca_w1, dtype=np.float32)
    ca_w2 = np.asarray(ca_w2, dtype=np.float32)

    # dynamic depthwise kernels + SE gate (tiny, fp32 host math)
    h = _lrelu(x1 @ W1.T)                                   # [B, 64]
    kern = (h @ W2.T).reshape(B, C, K)                      # [B, C, K]
    att = 1.0 / (1.0 + np.exp(-(_lrelu(x1 @ ca_w1.T) @ ca_w2.T)))
    att = att.astype(np.float32)                            # [B, C]

    # block-diagonal 1x1-conv weight as lhsT: lhsT[k, m] = W[m, k]
    wblk_np = np.zeros((P, P), np.float32)
    wblk_np[:C, :C] = conv_w.T
    wblk_np[C:, C:] = conv_w.T
    wblk_np = wblk_np.astype(BF16_NP)

    key = (USE_LRELU,)
    if key not in _COMPILED:
        _COMPILED[key] = _build_program(USE_LRELU)
    nc = _COMPILED[key]

    biasP = np.tile(conv_b, 2).astype(np.float32)            # [P]
    in_maps = []
    for core in range(N_CORES):
        s0 = core * SAMPLES_PER_CORE
        diags_np = np.zeros((P, PAIRS * K * P), np.float32)
        scal_np = np.empty((P, 2 * PAIRS), np.float32)
        in_map = {}
        for p in range(PAIRS):
            ka = kern[s0 + 2 * p]          # [C, K]
            kb = kern[s0 + 2 * p + 1]
            kern_bf = np.empty((P, K), np.float32)
            for j in range(K):
                s = (p * K + j) * P
                d = np.concatenate([ka[:, j], kb[:, j]])
                np.fill_diagonal(diags_np[:, s:s + P], d)
                kern_bf[:, j] = d.astype(BF16_NP).astype(np.float32)
            attp = np.concatenate([att[s0 + 2 * p], att[s0 + 2 * p + 1]])
            dp = biasP / attp                                 # [P]
            scal_np[:, p] = attp
            # depthwise compensation: -sum_j bf16(kern_j) * d
            scal_np[:, PAIRS + p] = -(kern_bf.sum(axis=1) * dp)
            # shifted input for this pair, with halo columns; edge halo = d
            xp = x0[s0 + 2 * p:s0 + 2 * p + 2].reshape(P, L) + dp[:, None]
            xp8 = np.empty((P, L + 2), BF16_NP)
            xp8[:, 1:L + 1] = xp.astype(BF16_NP)
            dp8 = dp.astype(BF16_NP)
            xp8[:, 0] = dp8
            xp8[:, L + 1] = dp8
            lo = 0
            for c, csz in enumerate(CHUNK_SIZES[p]):
                in_map[f"xin_{p}_{c}"] = np.ascontiguousarray(
                    xp8[:, lo:lo + csz + 2])
                lo += csz
        in_map["diags"] = diags_np.astype(BF16_NP)
        in_map["scal"] = scal_np
        in_map["wblk"] = wblk_np
        in_maps.append(in_map)

    res = run_bass_kernel_spmd(nc, in_maps, list(range(N_CORES)), trace=TRACE)
    LAST_RESULT = res

    full = np.empty((B, C, L), np.float32)
    for core in range(N_CORES):
        s0 = core * SAMPLES_PER_CORE
        r = res.results[core]
        for p in range(PAIRS):
            cols = np.concatenate(
                [r[f"out_{p}_{c}"] for c in range(len(CHUNK_SIZES[p]))],
                axis=1)
            full[s0 + 2 * p:s0 + 2 * p + 2] = (
                cols.reshape(2, C, L).astype(np.float32))
    return full
